# revision 5
# baseline (speedup 1.0000x reference)
"""Multi-head attention (B=2, S=2048, D=1024, H=16) on 8 Trainium2 NeuronCores.

Sharding: data-parallel over batch (2) x tensor-parallel over head groups
(16 heads -> 4 groups of 4). Core c handles batch b = c//4, head group
g = c%4 (output dims g*256:(g+1)*256 of the QKV projections).

Each core computes, fully on-device:
  qT, kT  [256, 2048]   (transposed projections, head-dim on partitions)
  v       [2048, 256]   (natural layout, +ones column per head for the
                         softmax denominator)
  scoresT[h] = kT_h^T-chunks x qT_h  -> exp (scaled 1/8) -> S'T (bf16)
  ytT[h]  = [v_h | 1]^T @ S'T  -> rows 0..63 = unnormalized attn out.T,
                                  row 64 = softmax denominator
  normalize rows by 1/denom, then partial output projection
  out_c [2048, 1024] = Y_g @ Wo[:, g*256:(g+1)*256].T   (fp32 partial)

Host sums the 4 partials per batch and adds bo.

All matmuls run in bf16 (fp32 PSUM accumulation). Host pre-transposes
activations/weights so the kernel needs no on-chip transposes.
"""

from contextlib import ExitStack

import numpy as np
import ml_dtypes

import concourse.bass as bass
import concourse.mybir as mybir
import concourse.tile as tile
from concourse import bacc
from concourse.bass import ts, ds
from concourse.bass_utils import run_bass_kernel_spmd

BF16 = mybir.dt.bfloat16
F32 = mybir.dt.float32
AF = mybir.ActivationFunctionType

S = 2048      # sequence length
D = 1024      # model dim
G = 256       # per-core projection width (4 heads x 64)
NH = 4        # heads per core
DH = 64       # head dim
NJ = 4        # sq blocks of 512
NSK = 16      # sk chunks of 128
NDK = 8       # d chunks of 128

_CACHE = {}
LAST_RESULTS = None


def _emit(tc, out_ap, xq, xk, xv, wq, wk, wv, wo, bias):
    nc = tc.nc
    ctx = ExitStack()

    cst = ctx.enter_context(tc.tile_pool(name="cst", bufs=1))
    xpool = ctx.enter_context(tc.tile_pool(name="xp", bufs=10))
    stp = ctx.enter_context(tc.tile_pool(name="stp", bufs=18))
    smalls = ctx.enter_context(tc.tile_pool(name="sm", bufs=2))
    obp = ctx.enter_context(tc.tile_pool(name="obp", bufs=2))
    pbig = ctx.enter_context(tc.tile_pool(name="pbig", bufs=2, space="PSUM"))
    pacc = ctx.enter_context(tc.tile_pool(name="pacc", bufs=2, space="PSUM"))
    pbc = ctx.enter_context(tc.tile_pool(name="pbc", bufs=1, space="PSUM"))

    # --- resident tensors -------------------------------------------------
    w_q = cst.tile([128, NDK, G], BF16, tag="w_q")
    w_k = cst.tile([128, NDK, G], BF16, tag="w_k")
    w_v = cst.tile([128, NDK, G], BF16, tag="w_v")
    w_o = cst.tile([64, NH, D], BF16, tag="w_o")
    bias_sb = cst.tile([1, 3 * G], BF16, tag="bias")
    ones_bf = cst.tile([1, 512], BF16, tag="ones_bf")
    ones_f32 = cst.tile([65, 64], F32, tag="ones_f32")
    qT = cst.tile([128, 2, S], BF16, tag="qT")
    kT = cst.tile([128, 2, S], BF16, tag="kT")
    v_aug = cst.tile([128, NSK, NH * 65], BF16, tag="v_aug")
    yt_lo = cst.tile([64, NH, S], BF16, tag="yt_lo")

    for dk in range(NDK):
        nc.sync.dma_start(w_q[:, dk, :], wq[ts(dk, 128), :])
        nc.sync.dma_start(w_k[:, dk, :], wk[ts(dk, 128), :])
        nc.sync.dma_start(w_v[:, dk, :], wv[ts(dk, 128), :])
    for h in range(NH):
        nc.sync.dma_start(w_o[:, h, :], wo[ts(h, 64), :])
    nc.sync.dma_start(bias_sb[:], bias[:])
    nc.vector.memset(ones_bf[:], 1.0)
    nc.vector.memset(ones_f32[:], 1.0)
    # ones columns of v_aug (col 64 of each head slab)
    nc.vector.memset(
        v_aug[:].rearrange("p m (h c) -> p m h c", c=65)[:, :, :, 64:65], 1.0
    )

    # --- phase A: projections --------------------------------------------
    # q, k in transposed layout [256, S]: pt p rows = heads 2p, 2p+1
    for idx, (xdram, wsb, dest) in enumerate(((xq, w_q, qT), (xk, w_k, kT))):
        for n in range(NJ):
            xt = []
            for dk in range(NDK):
                t = xpool.tile([128, 512], BF16, tag="x")
                nc.sync.dma_start(t[:], xdram[ts(dk, 128), ts(n, 512)])
                xt.append(t)
            ps = pbig.tile([128, 1024], F32, tag="sc")
            for p in range(2):
                sl = ps[:, ts(p, 512)]
                for dk in range(NDK):
                    nc.tensor.matmul(
                        sl, wsb[:, dk, ds(p * 128, 128)], xt[dk][:],
                        start=(dk == 0), stop=False,
                    )
                nc.tensor.matmul(
                    sl, bias_sb[:, ds(idx * G + p * 128, 128)],
                    ones_bf[:, 0:512], start=False, stop=True,
                )
                nc.vector.tensor_copy(dest[:, p, ts(n, 512)], sl)

    # v in natural layout [S, 256], interleaved with ones columns
    for mg in range(NJ):
        xt = []
        for dk in range(NDK):
            t = xpool.tile([128, 512], BF16, tag="x")
            nc.sync.dma_start(t[:], xv[ts(dk, 128), ts(mg, 512)])
            xt.append(t)
        for mi in range(4):
            m = mg * 4 + mi
            ps = pbig.tile([128, 1024], F32, tag="sc")
            sl = ps[:, 0:G]
            for dk in range(NDK):
                nc.tensor.matmul(
                    sl, xt[dk][:, ts(mi, 128)], w_v[:, dk, :],
                    start=(dk == 0), stop=False,
                )
            nc.tensor.matmul(
                sl, ones_bf[:, 0:128], bias_sb[:, ds(2 * G, G)],
                start=False, stop=True,
            )
            nc.vector.tensor_copy(
                v_aug[:, m].rearrange("p (h c) -> p h c", c=65)[:, :, 0:64],
                sl.rearrange("p (h c) -> p h c", c=64),
            )

    # --- phase B: attention (head pairs share partition dim) -------------
    for pair in range(2):
        for j in range(NJ):
            sts = []
            for i in range(NSK):
                sc = pbig.tile([128, 1024], F32, tag="sc")
                for hh in range(2):
                    off = hh * 64
                    nc.tensor.matmul(
                        sc[:, ts(hh, 512)],
                        kT[ds(off, 64), pair, ts(i, 128)],
                        qT[ds(off, 64), pair, ts(j, 512)],
                        start=True, stop=True,
                    )
                st = stp.tile([128, 1024], BF16, tag="st")
                nc.scalar.activation(st[:], sc[:], AF.Exp, scale=0.125)
                sts.append(st)
            for hh in range(2):
                h = 2 * pair + hh
                yt = pacc.tile([65, 512], F32, tag="yt")
                for i in range(NSK):
                    nc.tensor.matmul(
                        yt[:], v_aug[:, i, ds(h * 65, 65)],
                        sts[i][:, ts(hh, 512)],
                        start=(i == 0), stop=(i == NSK - 1),
                        skip_group_check=True,
                    )
                r = smalls.tile([65, 512], F32, tag="r")
                nc.vector.reciprocal(r[64:65, :], yt[64:65, :])
                rbp = pbc.tile([64, 512], F32, tag="rb")
                nc.tensor.matmul(
                    rbp[:], ones_f32[64:65, :], r[64:65, :],
                    start=True, stop=True,
                )
                rb = smalls.tile([64, 512], F32, tag="rbs")
                nc.vector.tensor_copy(rb[:], rbp[:])
                nc.vector.tensor_mul(
                    yt_lo[:, h, ts(j, 512)], yt[0:64, :], rb[:]
                )

    # --- phase C: output projection (partial) ----------------------------
    for m in range(NSK):
        po = pbig.tile([128, 1024], F32, tag="sc")
        for n2 in range(2):
            sl = po[:, ts(n2, 512)]
            for h in range(NH):
                nc.tensor.matmul(
                    sl, yt_lo[:, h, ts(m, 128)], w_o[:, h, ts(n2, 512)],
                    start=(h == 0), stop=(h == NH - 1),
                )
        ob = obp.tile([128, 1024], F32, tag="ob")
        nc.vector.tensor_copy(ob[:], po[:])
        nc.sync.dma_start(out_ap[ts(m, 128), :], ob[:])

    ctx.close()


def _build():
    if "nc" in _CACHE:
        return _CACHE["nc"]
    nc = bacc.Bacc("TRN2", target_bir_lowering=False, debug=False, num_devices=8)
    xq = nc.dram_tensor("xqT", [D, S], BF16, kind="ExternalInput").ap()
    xk = nc.dram_tensor("xkT", [D, S], BF16, kind="ExternalInput").ap()
    xv = nc.dram_tensor("xvT", [D, S], BF16, kind="ExternalInput").ap()
    wq = nc.dram_tensor("wqT", [D, G], BF16, kind="ExternalInput").ap()
    wk = nc.dram_tensor("wkT", [D, G], BF16, kind="ExternalInput").ap()
    wv = nc.dram_tensor("wvT", [D, G], BF16, kind="ExternalInput").ap()
    wo = nc.dram_tensor("woT", [G, D], BF16, kind="ExternalInput").ap()
    bias = nc.dram_tensor("bias", [1, 3 * G], BF16, kind="ExternalInput").ap()
    out = nc.dram_tensor("out", [S, D], F32, kind="ExternalOutput").ap()
    with tile.TileContext(nc) as tc:
        _emit(tc, out, xq, xk, xv, wq, wk, wv, wo, bias)
    nc.compile()
    _CACHE["nc"] = nc
    return nc


def _bf16(x):
    return np.ascontiguousarray(x).astype(ml_dtypes.bfloat16)


def kernel(Q, K, V, Wq, bq, Wk, bk, Wv, bv, Wo, bo):
    global LAST_RESULTS
    Q, K, V = (np.asarray(a, np.float32) for a in (Q, K, V))
    Wq, Wk, Wv, Wo = (np.asarray(a, np.float32) for a in (Wq, Wk, Wv, Wo))
    bq, bk, bv, bo = (np.asarray(a, np.float32) for a in (bq, bk, bv, bo))

    nc = _build()
    in_maps = []
    for c in range(8):
        b, g = c // 4, c % 4
        gs = slice(g * G, (g + 1) * G)
        in_maps.append({
            "xqT": _bf16(Q[b].T),
            "xkT": _bf16(K[b].T),
            "xvT": _bf16(V[b].T),
            "wqT": _bf16(Wq[gs, :].T),
            "wkT": _bf16(Wk[gs, :].T),
            "wvT": _bf16(Wv[gs, :].T),
            "woT": _bf16(Wo[:, gs].T),
            "bias": _bf16(
                np.concatenate([bq[gs], bk[gs], bv[gs]])[None, :]
            ),
        })

    try:
        res = run_bass_kernel_spmd(nc, in_maps, core_ids=list(range(8)))
    except ModuleNotFoundError:
        # NTFF profiling hook unavailable in this container; run untraced.
        import os
        os.environ["BASS_NEVER_TRACE"] = "1"
        try:
            res = run_bass_kernel_spmd(nc, in_maps, core_ids=list(range(8)))
        finally:
            del os.environ["BASS_NEVER_TRACE"]
    LAST_RESULTS = res

    out = np.empty((2, S, D), np.float32)
    for b in range(2):
        acc = res.results[b * 4 + 0]["out"].astype(np.float32).copy()
        for g in range(1, 4):
            acc += res.results[b * 4 + g]["out"]
        out[b] = acc + bo[None, :]
    return out


# revision 24
# speedup vs baseline: 2.1355x; 2.1355x over previous
"""Multi-head attention (B=2, S=2048, D=1024, H=16) on 8 Trainium2 NeuronCores.

Sharding: data-parallel over batch (2) x tensor-parallel over head groups
(16 heads -> 4 groups of 4). Core c handles batch b = c//4, head group
g = c%4 (output dims g*256:(g+1)*256 of the QKV projections).

Each core computes, fully on-device:
  qT, kT  [256, 2048]   (transposed projections, head-dim on partitions)
  v       [2048, 256]   (natural layout, +ones column per head for the
                         softmax denominator)
  scoresT[h] = kT_h^T-chunks x qT_h  -> exp (scaled 1/8) -> S'T (bf16)
  ytT[h]  = [v_h | 1]^T @ S'T  -> rows 0..63 = unnormalized attn out.T,
                                  row 64 = softmax denominator
  normalize rows by 1/denom, then partial output projection
  out_c [2048, 1024] = Y_g @ Wo[:, g*256:(g+1)*256].T   (fp32 partial)

Host sums the 4 partials per batch and adds bo.

All matmuls run in bf16 (fp32 PSUM accumulation). Host pre-transposes
activations/weights so the kernel needs no on-chip transposes.
"""

from contextlib import ExitStack

import numpy as np
import ml_dtypes

import concourse.bass as bass
import concourse.mybir as mybir
import concourse.tile as tile
from concourse import bacc
from concourse.bass import ts, ds
from concourse.bass_utils import run_bass_kernel_spmd

BF16 = mybir.dt.bfloat16
F32 = mybir.dt.float32
F32R = mybir.dt.float32r
AF = mybir.ActivationFunctionType

S = 2048      # sequence length
D = 1024      # model dim
G = 256       # per-core projection width (4 heads x 64)
NH = 4        # heads per core
DH = 64       # head dim
NJ = 4        # sq blocks of 512
NSK = 16      # sk chunks of 128
NDK = 8       # d chunks of 128

_CACHE = {}
LAST_RESULTS = None


def _emit(tc, out_ap, xq, xk, xv, wq, wk, wv, wo, bias):
    nc = tc.nc
    ctx = ExitStack()

    cst = ctx.enter_context(tc.tile_pool(name="cst", bufs=1))
    xpool = ctx.enter_context(tc.tile_pool(name="xp", bufs=16))
    stp = ctx.enter_context(tc.tile_pool(name="stp", bufs=18))
    smalls = ctx.enter_context(tc.tile_pool(name="sm", bufs=2))
    obp = ctx.enter_context(tc.tile_pool(name="obp", bufs=2))
    pbig = ctx.enter_context(tc.tile_pool(name="pbig", bufs=2, space="PSUM"))
    pacc = ctx.enter_context(tc.tile_pool(name="pacc", bufs=2, space="PSUM"))
    pbc = ctx.enter_context(tc.tile_pool(name="pbc", bufs=1, space="PSUM"))

    # --- resident tensors -------------------------------------------------
    w_q = cst.tile([128, NDK, G], BF16, tag="w_q")
    w_k = cst.tile([128, NDK, G], BF16, tag="w_k")
    w_v = cst.tile([128, NDK, G], BF16, tag="w_v")
    w_o = cst.tile([128, 2, D], BF16, tag="w_o")
    bias_sb = cst.tile([1, 3 * G], BF16, tag="bias")
    ones_bf = cst.tile([1, 512], BF16, tag="ones_bf")
    ones_f32 = cst.tile([65, 64], F32R, tag="ones_f32")
    ones_tmp = cst.tile([65, 64], F32, tag="ones_tmp")
    qT = [
        cst.tile([128, S], BF16, tag=f"qT{p}", name=f"qT{p}") for p in range(2)
    ]
    kT = [
        cst.tile([128, S], BF16, tag=f"kT{p}", name=f"kT{p}") for p in range(2)
    ]
    v_aug = cst.tile([128, NSK, NH * 65], BF16, tag="v_aug")
    yt_sb = [
        cst.tile([128, 2, 512], BF16, tag=f"yt{j}", name=f"yt{j}")
        for j in range(NJ)
    ]

    for dk in range(NDK):
        nc.sync.dma_start(w_q[:, dk, :], wq[ts(dk, 128), :])
        nc.sync.dma_start(w_k[:, dk, :], wk[ts(dk, 128), :])
        nc.sync.dma_start(w_v[:, dk, :], wv[ts(dk, 128), :])
    for p in range(2):
        nc.sync.dma_start(w_o[:, p, :], wo[ts(p, 128), :])
    nc.sync.dma_start(bias_sb[:], bias[:])
    nc.vector.memset(ones_bf[:], 1.0)
    nc.vector.memset(ones_tmp[:], 1.0)
    nc.vector.tensor_copy(ones_f32[:], ones_tmp[:])
    # ones columns of v_aug (col 64 of each head slab)
    nc.vector.memset(
        v_aug[:].rearrange("p m (h c) -> p m h c", c=65)[:, :, :, 64:65], 1.0
    )

    # --- phase A: projections --------------------------------------------
    def load_x(xdram):
        xt = []
        for dk in range(NDK):
            t = xpool.tile([128, S], BF16, tag="x")
            nc.sync.dma_start(t[:], xdram[ts(dk, 128), :])
            xt.append(t)
        return xt

    # v in natural layout [S, 256], interleaved with ones columns
    xvt = load_x(xv)
    for m in range(NSK):
        ps = pbig.tile([128, 1024], F32, tag="sc")
        sl = ps[:, 0:G]
        for dk in range(NDK):
            nc.tensor.matmul(
                sl, xvt[dk][:, ts(m, 128)], w_v[:, dk, :],
                start=(dk == 0), stop=False,
            )
        nc.tensor.matmul(
            sl, ones_bf[:, 0:128], bias_sb[:, ds(2 * G, G)],
            start=False, stop=True,
        )
        nc.scalar.copy(
            v_aug[:, m].rearrange("p (h c) -> p h c", c=65)[:, :, 0:64],
            sl.rearrange("p (h c) -> p h c", c=64),
        )

    # q, k in transposed layout [256, S]: pt p rows = heads 2p, 2p+1
    xqt = load_x(xq)
    xkt = load_x(xk)
    for p in range(2):
        for idx, (xt, wsb, dest) in enumerate(
            ((xqt, w_q, qT[p]), (xkt, w_k, kT[p]))
        ):
            for ng in range(2):
                ps = pbig.tile([128, 1024], F32, tag="sc")
                for ni in range(2):
                    n = ng * 2 + ni
                    sl = ps[:, ts(ni, 512)]
                    for dk in range(NDK):
                        nc.tensor.matmul(
                            sl, wsb[:, dk, ds(p * 128, 128)],
                            xt[dk][:, ts(n, 512)],
                            start=(dk == 0), stop=False,
                        )
                    nc.tensor.matmul(
                        sl, bias_sb[:, ds(idx * G + p * 128, 128)],
                        ones_bf[:, 0:512], start=False, stop=True,
                    )
                nc.scalar.copy(dest[:, ts(ng, 1024)], ps[:])

    # --- phase B: attention (head pairs share partition dim) -------------
    for pair in range(2):
        for j in range(NJ):
            sts = []
            for i in range(NSK):
                sc = pbig.tile([128, 1024], F32, tag="sc")
                for hh in range(2):
                    off = hh * 64
                    nc.tensor.matmul(
                        sc[:, ts(hh, 512)],
                        kT[pair][ds(off, 64), ts(i, 128)],
                        qT[pair][ds(off, 64), ts(j, 512)],
                        start=True, stop=True,
                    )
                st = stp.tile([128, 1024], BF16, tag="st")
                nc.scalar.activation(st[:], sc[:], AF.Exp, scale=0.125)
                sts.append(st)
            for hh in range(2):
                h = 2 * pair + hh
                yt = pacc.tile([65, 512], F32, tag="yt")
                for i in range(NSK):
                    nc.tensor.matmul(
                        yt[:], v_aug[:, i, ds(h * 65, 65)],
                        sts[i][:, ts(hh, 512)],
                        start=(i == 0), stop=(i == NSK - 1),
                        skip_group_check=True,
                    )
                rf = smalls.tile([65, 512], F32, tag="rf")
                nc.vector.reciprocal(rf[64:65, :], yt[64:65, :])
                r = smalls.tile([65, 512], F32R, tag="r")
                nc.vector.tensor_copy(r[64:65, :], rf[64:65, :])
                rbp = pbc.tile([64, 512], F32, tag="rb")
                nc.tensor.matmul(
                    rbp[:], ones_f32[64:65, :], r[64:65, :],
                    start=True, stop=True,
                )
                rb = smalls.tile([64, 512], F32, tag="rbs")
                nc.vector.tensor_copy(rb[:], rbp[:])
                nc.vector.tensor_mul(
                    yt_sb[j][ds(hh * 64, 64), pair, :], yt[0:64, :], rb[:]
                )

    # --- phase C: output projection (partial) ----------------------------
    for m in range(NSK):
        po = pbig.tile([128, 1024], F32, tag="sc")
        for n2 in range(2):
            sl = po[:, ts(n2, 512)]
            for p in range(2):
                nc.tensor.matmul(
                    sl, yt_sb[m // 4][:, p, ds((m % 4) * 128, 128)],
                    w_o[:, p, ts(n2, 512)],
                    start=(p == 0), stop=(p == 1),
                )
        ob = obp.tile([128, 1024], BF16, tag="ob")
        nc.scalar.copy(ob[:], po[:])
        nc.sync.dma_start(out_ap[ts(m, 128), :], ob[:])

    ctx.close()


def _build(reps=1):
    key = ("nc", reps)
    if key in _CACHE:
        return _CACHE[key]
    nc = bacc.Bacc("TRN2", target_bir_lowering=False, debug=False, num_devices=8)
    xq = nc.dram_tensor("xqT", [D, S], BF16, kind="ExternalInput").ap()
    xk = nc.dram_tensor("xkT", [D, S], BF16, kind="ExternalInput").ap()
    xv = nc.dram_tensor("xvT", [D, S], BF16, kind="ExternalInput").ap()
    wq = nc.dram_tensor("wqT", [D, G], BF16, kind="ExternalInput").ap()
    wk = nc.dram_tensor("wkT", [D, G], BF16, kind="ExternalInput").ap()
    wv = nc.dram_tensor("wvT", [D, G], BF16, kind="ExternalInput").ap()
    wo = nc.dram_tensor("woT", [G, D], BF16, kind="ExternalInput").ap()
    bias = nc.dram_tensor("bias", [1, 3 * G], BF16, kind="ExternalInput").ap()
    out = nc.dram_tensor("out", [S, D], BF16, kind="ExternalOutput").ap()
    with tile.TileContext(nc) as tc:
        for _ in range(reps):
            _emit(tc, out, xq, xk, xv, wq, wk, wv, wo, bias)
    nc.compile()
    _CACHE[key] = nc
    return nc


def _bf16(x):
    return np.ascontiguousarray(x).astype(ml_dtypes.bfloat16)


def kernel(Q, K, V, Wq, bq, Wk, bk, Wv, bv, Wo, bo):
    global LAST_RESULTS
    Q, K, V = (np.asarray(a, np.float32) for a in (Q, K, V))
    Wq, Wk, Wv, Wo = (np.asarray(a, np.float32) for a in (Wq, Wk, Wv, Wo))
    bq, bk, bv, bo = (np.asarray(a, np.float32) for a in (bq, bk, bv, bo))

    nc = _build()
    in_maps = []
    for c in range(8):
        b, g = c // 4, c % 4
        gs = slice(g * G, (g + 1) * G)
        in_maps.append({
            "xqT": _bf16(Q[b].T),
            "xkT": _bf16(K[b].T),
            "xvT": _bf16(V[b].T),
            "wqT": _bf16(Wq[gs, :].T),
            "wkT": _bf16(Wk[gs, :].T),
            "wvT": _bf16(Wv[gs, :].T),
            "woT": _bf16(Wo[:, gs].T),
            "bias": _bf16(
                np.concatenate([bq[gs], bk[gs], bv[gs]])[None, :]
            ),
        })

    try:
        res = run_bass_kernel_spmd(nc, in_maps, core_ids=list(range(8)))
    except ModuleNotFoundError:
        # NTFF profiling hook unavailable in this container; run untraced.
        import os
        os.environ["BASS_NEVER_TRACE"] = "1"
        try:
            res = run_bass_kernel_spmd(nc, in_maps, core_ids=list(range(8)))
        finally:
            del os.environ["BASS_NEVER_TRACE"]
    LAST_RESULTS = res

    out = np.empty((2, S, D), np.float32)
    for b in range(2):
        acc = res.results[b * 4 + 0]["out"].astype(np.float32).copy()
        for g in range(1, 4):
            acc += res.results[b * 4 + g]["out"]
        out[b] = acc + bo[None, :]
    return out


# revision 30
# speedup vs baseline: 2.5890x; 1.2123x over previous
"""Multi-head attention (B=2, S=2048, D=1024, H=16) on 8 Trainium2 NeuronCores.

Sharding: data-parallel over batch (2) x tensor-parallel over head groups
(16 heads -> 4 groups of 4). Core c handles batch b = c//4, head group
g = c%4 (output dims g*256:(g+1)*256 of the QKV projections).

Each core computes, fully on-device:
  qT, kT  [256, 2048]   (transposed projections, head-dim on partitions)
  v       [2048, 256]   (natural layout, +ones column per head for the
                         softmax denominator)
  scoresT[h] = kT_h^T-chunks x qT_h  -> exp (scaled 1/8) -> S'T (bf16)
  ytT[h]  = [v_h | 1]^T @ S'T  -> rows 0..63 = unnormalized attn out.T,
                                  row 64 = softmax denominator
  normalize rows by 1/denom, then partial output projection
  out_c [2048, 1024] = Y_g @ Wo[:, g*256:(g+1)*256].T   (bf16 partial)

Host sums the 4 partials per batch and adds bo.

All matmuls run in bf16 (fp32 PSUM accumulation). Host pre-transposes
activations/weights so the kernel needs no on-chip transposes.
"""

from contextlib import ExitStack

import numpy as np
import ml_dtypes

import concourse.bass as bass
import concourse.mybir as mybir
import concourse.tile as tile
from concourse import bacc
from concourse.bass import ts, ds
from concourse.bass_utils import run_bass_kernel_spmd

BF16 = mybir.dt.bfloat16
F32 = mybir.dt.float32
F32R = mybir.dt.float32r
AF = mybir.ActivationFunctionType

S = 2048      # sequence length
D = 1024      # model dim
G = 256       # per-core projection width (4 heads x 64)
NH = 4        # heads per core
DH = 64       # head dim
NJ = 4        # sq blocks of 512
NSK = 16      # sk chunks of 128
NDK = 8       # d chunks of 128

_CACHE = {}
LAST_RESULTS = None


def _emit(tc, out_ap, xq, xk, xv, wq, wk, wv, wo, bias, phases="abc"):
    nc = tc.nc
    ctx = ExitStack()

    cst = ctx.enter_context(tc.tile_pool(name="cst", bufs=1))
    xpool = ctx.enter_context(tc.tile_pool(name="xp", bufs=16))
    stp = ctx.enter_context(tc.tile_pool(name="stp", bufs=18))
    smalls = ctx.enter_context(tc.tile_pool(name="sm", bufs=2))
    obp = ctx.enter_context(tc.tile_pool(name="obp", bufs=2))
    pbig = ctx.enter_context(tc.tile_pool(name="pbig", bufs=2, space="PSUM"))
    pacc = ctx.enter_context(tc.tile_pool(name="pacc", bufs=2, space="PSUM"))
    pbc = ctx.enter_context(tc.tile_pool(name="pbc", bufs=1, space="PSUM"))

    # --- resident tensors -------------------------------------------------
    w_q = cst.tile([128, NDK, G], BF16, tag="w_q")
    w_k = cst.tile([128, NDK, G], BF16, tag="w_k")
    w_v = cst.tile([128, NDK, G], BF16, tag="w_v")
    w_o = cst.tile([128, 2, D], BF16, tag="w_o")
    bias_sb = cst.tile([1, 3 * G], BF16, tag="bias")
    ones_bf = cst.tile([1, 512], BF16, tag="ones_bf")
    ones_f32 = cst.tile([65, 64], F32R, tag="ones_f32")
    ones_tmp = cst.tile([65, 64], F32, tag="ones_tmp")
    qT = [
        cst.tile([128, S], BF16, tag=f"qT{p}", name=f"qT{p}") for p in range(2)
    ]
    kT = [
        cst.tile([128, S], BF16, tag=f"kT{p}", name=f"kT{p}") for p in range(2)
    ]
    v_aug = cst.tile([128, NSK, NH * 65], BF16, tag="v_aug")
    yt_sb = [
        cst.tile([128, 2, 512], BF16, tag=f"yt{j}", name=f"yt{j}")
        for j in range(NJ)
    ]

    for dk in range(NDK):
        nc.sync.dma_start(w_q[:, dk, :], wq[ts(dk, 128), :])
        nc.sync.dma_start(w_k[:, dk, :], wk[ts(dk, 128), :])
        nc.sync.dma_start(w_v[:, dk, :], wv[ts(dk, 128), :])
    for p in range(2):
        nc.sync.dma_start(w_o[:, p, :], wo[ts(p, 128), :])
    nc.sync.dma_start(bias_sb[:], bias[:])
    nc.vector.memset(ones_bf[:], 1.0)
    nc.vector.memset(ones_tmp[:], 1.0)
    nc.vector.tensor_copy(ones_f32[:], ones_tmp[:])
    # ones columns of v_aug (col 64 of each head slab)
    nc.vector.memset(
        v_aug[:].rearrange("p m (h c) -> p m h c", c=65)[:, :, :, 64:65], 1.0
    )

    # --- phase A: projections --------------------------------------------
    def load_x(xdram):
        xt = []
        for dk in range(NDK):
            t = xpool.tile([128, S], BF16, tag="x")
            nc.sync.dma_start(t[:], xdram[ts(dk, 128), :])
            xt.append(t)
        return xt

    # v in natural layout [S, 256], interleaved with ones columns
    xvt = load_x(xv)
    for m in range(NSK):
        ps = pbig.tile([128, 1024], F32, tag="sc")
        sl = ps[:, 0:G]
        for dk in range(NDK):
            nc.tensor.matmul(
                sl, xvt[dk][:, ts(m, 128)], w_v[:, dk, :],
                start=(dk == 0), stop=False,
            )
        nc.tensor.matmul(
            sl, ones_bf[:, 0:128], bias_sb[:, ds(2 * G, G)],
            start=False, stop=True,
        )
        nc.scalar.copy(
            v_aug[:, m].rearrange("p (h c) -> p h c", c=65)[:, :, 0:64],
            sl.rearrange("p (h c) -> p h c", c=64),
        )

    # q, k in transposed layout [256, S]: pt p rows = heads 2p, 2p+1
    xqt = load_x(xq)
    xkt = load_x(xk)
    for p in range(2):
        for idx, (xt, wsb, dest) in enumerate(
            ((xqt, w_q, qT[p]), (xkt, w_k, kT[p]))
        ):
            for ng in range(2):
                ps = pbig.tile([128, 1024], F32, tag="sc")
                for ni in range(2):
                    n = ng * 2 + ni
                    sl = ps[:, ts(ni, 512)]
                    for dk in range(NDK):
                        nc.tensor.matmul(
                            sl, wsb[:, dk, ds(p * 128, 128)],
                            xt[dk][:, ts(n, 512)],
                            start=(dk == 0), stop=False,
                        )
                    nc.tensor.matmul(
                        sl, bias_sb[:, ds(idx * G + p * 128, 128)],
                        ones_bf[:, 0:512], start=False, stop=True,
                    )
                nc.scalar.copy(dest[:, ts(ng, 1024)], ps[:])

    # --- phase B: attention (head pairs share partition dim) -------------
    if "b" not in phases:
        # keep phase A live: dump projections to out
        nc.sync.dma_start(out_ap[0:128, :], qT[0][:, 0:1024])
        nc.sync.dma_start(out_ap[128:256, :], kT[1][:, 0:1024])
        nc.sync.dma_start(out_ap[256:384, 0:780], v_aug[:, 0:3, :].rearrange("p m c -> p (m c)"))
        ctx.close()
        return
    for pair in range(2):
        for j in range(NJ):
            sts = []
            for i in range(NSK):
                sc = pbig.tile([128, 1024], F32, tag="sc")
                for hh in range(2):
                    off = hh * 64
                    nc.tensor.matmul(
                        sc[:, ts(hh, 512)],
                        kT[pair][ds(off, 64), ts(i, 128)],
                        qT[pair][ds(off, 64), ts(j, 512)],
                        start=True, stop=True,
                    )
                st = stp.tile([128, 1024], BF16, tag="st")
                nc.scalar.activation(st[:], sc[:], AF.Exp, scale=0.125)
                sts.append(st)
            for hh in range(2):
                h = 2 * pair + hh
                yt = pacc.tile([65, 512], F32, tag="yt")
                for i in range(NSK):
                    nc.tensor.matmul(
                        yt[:], v_aug[:, i, ds(h * 65, 65)],
                        sts[i][:, ts(hh, 512)],
                        start=(i == 0), stop=(i == NSK - 1),
                        skip_group_check=True,
                    )
                rf = smalls.tile([65, 512], F32, tag="rf")
                nc.vector.reciprocal(rf[64:65, :], yt[64:65, :])
                r = smalls.tile([65, 512], F32R, tag="r")
                nc.vector.tensor_copy(r[64:65, :], rf[64:65, :])
                rbp = pbc.tile([64, 512], F32, tag="rb")
                nc.tensor.matmul(
                    rbp[:], ones_f32[64:65, :], r[64:65, :],
                    start=True, stop=True,
                )
                rb = smalls.tile([64, 512], F32, tag="rbs")
                nc.vector.tensor_copy(rb[:], rbp[:])
                nc.vector.tensor_mul(
                    yt_sb[j][ds(hh * 64, 64), pair, :], yt[0:64, :], rb[:]
                )

    # --- phase C: output projection (partial) ----------------------------
    if "c" not in phases:
        for j in range(NJ):
            nc.sync.dma_start(
                out_ap[ts(j, 128), :],
                yt_sb[j][:].rearrange("p a c -> p (a c)"),
            )
        ctx.close()
        return
    for m in range(NSK):
        po = pbig.tile([128, 1024], F32, tag="sc")
        for n2 in range(2):
            sl = po[:, ts(n2, 512)]
            for p in range(2):
                nc.tensor.matmul(
                    sl, yt_sb[m // 4][:, p, ds((m % 4) * 128, 128)],
                    w_o[:, p, ts(n2, 512)],
                    start=(p == 0), stop=(p == 1),
                )
        ob = obp.tile([128, 1024], BF16, tag="ob")
        nc.vector.tensor_copy(ob[:], po[:])
        nc.sync.dma_start(out_ap[ts(m, 128), :], ob[:])

    ctx.close()


def _build(reps=1, phases="abc"):
    key = ("nc", reps, phases)
    if key in _CACHE:
        return _CACHE[key]
    nc = bacc.Bacc("TRN2", target_bir_lowering=False, debug=False, num_devices=8)
    xq = nc.dram_tensor("xqT", [D, S], BF16, kind="ExternalInput").ap()
    xk = nc.dram_tensor("xkT", [D, S], BF16, kind="ExternalInput").ap()
    xv = nc.dram_tensor("xvT", [D, S], BF16, kind="ExternalInput").ap()
    wq = nc.dram_tensor("wqT", [D, G], BF16, kind="ExternalInput").ap()
    wk = nc.dram_tensor("wkT", [D, G], BF16, kind="ExternalInput").ap()
    wv = nc.dram_tensor("wvT", [D, G], BF16, kind="ExternalInput").ap()
    wo = nc.dram_tensor("woT", [G, D], BF16, kind="ExternalInput").ap()
    bias = nc.dram_tensor("bias", [1, 3 * G], BF16, kind="ExternalInput").ap()
    out = nc.dram_tensor("out", [S, D], BF16, kind="ExternalOutput").ap()
    with tile.TileContext(nc) as tc:
        for _ in range(reps):
            _emit(tc, out, xq, xk, xv, wq, wk, wv, wo, bias, phases=phases)
    nc.compile()
    _CACHE[key] = nc
    return nc


def _bf16(x):
    return np.ascontiguousarray(x).astype(ml_dtypes.bfloat16)


def kernel(Q, K, V, Wq, bq, Wk, bk, Wv, bv, Wo, bo):
    global LAST_RESULTS
    Q, K, V = (np.asarray(a, np.float32) for a in (Q, K, V))
    Wq, Wk, Wv, Wo = (np.asarray(a, np.float32) for a in (Wq, Wk, Wv, Wo))
    bq, bk, bv, bo = (np.asarray(a, np.float32) for a in (bq, bk, bv, bo))

    nc = _build()
    in_maps = []
    for c in range(8):
        b, g = c // 4, c % 4
        gs = slice(g * G, (g + 1) * G)
        in_maps.append({
            "xqT": _bf16(Q[b].T),
            "xkT": _bf16(K[b].T),
            "xvT": _bf16(V[b].T),
            "wqT": _bf16(Wq[gs, :].T),
            "wkT": _bf16(Wk[gs, :].T),
            "wvT": _bf16(Wv[gs, :].T),
            "woT": _bf16(Wo[:, gs].T),
            "bias": _bf16(
                np.concatenate([bq[gs], bk[gs], bv[gs]])[None, :]
            ),
        })

    try:
        res = run_bass_kernel_spmd(nc, in_maps, core_ids=list(range(8)))
    except ModuleNotFoundError:
        # NTFF profiling hook unavailable in this container; run untraced.
        import os
        os.environ["BASS_NEVER_TRACE"] = "1"
        try:
            res = run_bass_kernel_spmd(nc, in_maps, core_ids=list(range(8)))
        finally:
            del os.environ["BASS_NEVER_TRACE"]
    LAST_RESULTS = res

    out = np.empty((2, S, D), np.float32)
    for b in range(2):
        acc = res.results[b * 4 + 0]["out"].astype(np.float32).copy()
        for g in range(1, 4):
            acc += res.results[b * 4 + g]["out"]
        out[b] = acc + bo[None, :]
    return out
